# revision 2
# baseline (speedup 1.0000x reference)
"""EquiLinearLayer TRN2 kernel.

out[b,s,j,y] = sum_{i,x,bl} weights[j,i,bl] * blade[bl,x,y] * mv[b,s,i,x]

Strategy: pre-contract weights x blade on host into K4[(i,x),(j,y)] (tiny),
then the device does one token-parallel GEMM  out[t, jy] = mv[t, ix] @ K4.
Data-parallel over the batch dim: core c handles batch element c
(4096 tokens each). K4 (4 MB) is replicated into SBUF on every core.

Per 128-token tile:
  - DMA load mv tile [128t, 1024ix] (fp32 bits, declared float32r)
  - PE transposes 8x [128,128] -> PSUM to build mvT [ix, t] (matmul needs
    the contraction dim on partitions)
  - DVE evacuates PSUM -> SBUF
  - 16 fp32r matmuls: out[t128, jy512] += mvT_k.T @ K4[k, jy-half]
  - DVE evacuates PSUM -> SBUF, DMA store

fp32r = TRN2's full-rate 4-byte matmul mode (~13-bit mantissa,
measured ~1.5e-4 relative error at K=1024 vs fp64; plain fp32 runs at
1/4 rate).
"""
import numpy as np

import concourse.bacc as bacc
import concourse.mybir as mybir
import concourse.tile as tile
from concourse import bass_utils

B, S = 8, 4096
IN_CH, OUT_CH = 64, 64
BLADES, MVD = 9, 16
N_CORES = 8

T = (B * S) // N_CORES        # tokens per core = 4096
IX = IN_CH * MVD              # 1024 contraction
JY = OUT_CH * MVD             # 1024 output features
TT = 128                      # token tile
NT = T // TT                  # 32 tiles per core
KC = IX // 128                # 8 contraction chunks
NJ = 2                        # jy halves of 512

f32 = mybir.dt.float32
f32r = mybir.dt.float32r

_NC_CACHE = {}


def build_module():
    if "nc" in _NC_CACHE:
        return _NC_CACHE["nc"]
    nc = bacc.Bacc("TRN2", target_bir_lowering=False, debug=False,
                   num_devices=N_CORES)
    mv_d = nc.dram_tensor("mv", [T, IX], f32r, kind="ExternalInput").ap()
    k4_d = nc.dram_tensor("k4", [IX, JY], f32r, kind="ExternalInput").ap()
    id_d = nc.dram_tensor("ident", [128, 128], f32r, kind="ExternalInput").ap()
    out_d = nc.dram_tensor("out", [T, JY], f32, kind="ExternalOutput").ap()

    with tile.TileContext(nc) as tc:
        with (
            tc.tile_pool(name="const", bufs=1) as cpool,
            tc.tile_pool(name="io", bufs=3) as iopool,
            tc.tile_pool(name="psum", bufs=2, space="PSUM") as pspool,
        ):
            k4_s = cpool.tile([128, KC * JY], f32r, tag="k4")
            id_s = cpool.tile([128, 128], f32r, tag="id")
            nc.sync.dma_start(
                k4_s[:].rearrange("p (kc j) -> p kc j", kc=KC),
                k4_d[:].rearrange("(kc p) j -> p kc j", p=128),
            )
            nc.sync.dma_start(id_s[:], id_d[:])

            for ti in range(NT):
                mv_s = iopool.tile([128, IX], f32r, tag="mv")
                nc.sync.dma_start(mv_s[:], mv_d[ti * TT:(ti + 1) * TT, :])

                ps_tr = pspool.tile([128, IX], f32r, tag="ptr")
                for kc in range(KC):
                    nc.tensor.transpose(
                        ps_tr[:, kc * 128:(kc + 1) * 128],
                        mv_s[:, kc * 128:(kc + 1) * 128],
                        id_s[:],
                    )
                mvt_s = iopool.tile([128, IX], f32r, tag="mvt")
                nc.vector.tensor_copy(mvt_s[:], ps_tr[:])

                pos = [pspool.tile([128, 512], f32, tag=f"po{jc}",
                                   name=f"po{jc}_{ti}")
                       for jc in range(NJ)]
                for kc in range(KC):
                    for jc in range(NJ):
                        nc.tensor.matmul(
                            pos[jc][:],
                            mvt_s[:, kc * 128:(kc + 1) * 128],
                            k4_s[:, kc * JY + jc * 512: kc * JY + jc * 512 + 512],
                            start=(kc == 0), stop=(kc == KC - 1),
                        )
                out_s = iopool.tile([128, JY], f32, tag="out")
                for jc in range(NJ):
                    nc.vector.tensor_copy(
                        out_s[:, jc * 512:(jc + 1) * 512], pos[jc][:])
                nc.sync.dma_start(out_d[ti * TT:(ti + 1) * TT, :], out_s[:])

    nc.compile()
    _NC_CACHE["nc"] = nc
    return nc


def make_in_maps(multivectors, weights, blade):
    k4 = np.einsum("jib,bxy->ixjy", weights, blade).reshape(IX, JY)
    k4 = np.ascontiguousarray(k4, dtype=np.float32)
    ident = np.eye(128, dtype=np.float32)
    mv = np.ascontiguousarray(multivectors, dtype=np.float32).reshape(B * S, IX)
    return [
        {"mv": mv[c * T:(c + 1) * T], "k4": k4, "ident": ident}
        for c in range(N_CORES)
    ]


def kernel(multivectors, weights, blade):
    nc = build_module()
    in_maps = make_in_maps(multivectors, weights, blade)
    r = bass_utils.run_bass_kernel_spmd(nc, in_maps, core_ids=list(range(N_CORES)))
    out = np.stack([r.results[c]["out"] for c in range(N_CORES)])
    return out.reshape(B, S, OUT_CH, MVD)
